# revision 65
# baseline (speedup 1.0000x reference)
"""Trainium2 Bass kernel for nn_BetaVAEMark10Decoder.

Network (per sample): latent(4) -> Linear(256)+leaky -> reshape (1,8,32)
 -> convT(5,2)s(5,2) -> conv3x3 SAME +leaky   (5,16,16)
 -> convT(5,2)s(5,2) -> conv3x3 SAME +leaky   (25,32,8)
 -> convT(2,2)s(2,2) -> conv3x3 SAME +relu    (50,64,6)  -> NCHW out.

Each convT(k=s) + 3x3 conv pair composes into one exact linear map that is
block-banded over rows: output row y reads input rows i+d through per-phase
matrices R[p, d].  Everything becomes dense matmuls on 128-chunks.

This version runs the whole stack in bf16 (1 cycle/row on the PE for any N,
vs fp32r's 4x penalty under N=256) and exploits the *x*-banded structure of
the final layer: with x-major output ordering (col = x*6 + o), each 128-col
group of an output row only reads a feature window of the input row
(j-window * 8 ch): feats 0..128 / 80..176 / 128..256.  The outer windows are
the two natural 128-chunks of x3; the middle one is stitched by a cheap
SBUF-to-SBUF DMA.  L4 then needs only 3 matmuls of N=128 per (row-contrib,
batch-block): 150k PE cycles instead of the 301k a dense 2-chunk
contraction costs.  The kernel is PE-bound at ~92% occupancy (~97us busy).

Other levers: bias folded into the L1 contraction as a 5th ones-row (no
bias DMA/path), PE p-state warm-up matmuls under the first weight DMA, a
single 8-bank psum pool of [128,2,512] pair tiles (4-deep, so the PE never
waits on relu latency), relu split across ACT and DVE (GPSIMD cannot read
PSUM), and bf16 output (half the DMA bytes -> ~55us on the 360GB/s bus;
upcast on host) flushed every other row-pair, staggered by batch-block
parity, so the last transfer is a short 2-row tile.

Sharding: pure data parallel, batch 4096 -> 8 cores x 512.
"""

import sys

import numpy as np

sys.path.insert(0, "/opt/trn_rl_repo")

import ml_dtypes  # noqa: E402

import concourse.bass as bass  # noqa: E402
import concourse.bacc as bacc  # noqa: E402
import concourse.mybir as mybir  # noqa: E402
from concourse import tile  # noqa: E402
from concourse.bass_utils import run_bass_kernel_spmd  # noqa: E402

N_CORES = 8
B = 4096
BL = B // N_CORES  # 512 per core
F32 = mybir.dt.float32
F32R = mybir.dt.float32r
BF16 = mybir.dt.bfloat16
NPBF = ml_dtypes.bfloat16

# L4 feature windows per column group: group g of an output row (cols
# 128g..128(g+1), x-major) only reads input feats within these windows.
L4_WIN = ((0, 128), (80, 176), (128, 256))


# ---------------------------------------------------------------- host math
def _fused_matrices(Wup, Wc, sy, sx, Win, in_idx, out_idx, n_out_cols):
    """Compose convT(k=s=(sy,sx)) with 3x3 SAME conv into per-phase row
    matrices.  Returns {(p, delta): M} where out row y (p = y%sy, i = y//sy)
    accumulates  in_row[i+delta] @ M[(p, delta)]  over available deltas.
    x-edge clipping is baked into M; y-edge clipping == skipping absent rows.
    """
    Wup = np.asarray(Wup, np.float32)
    Wc = np.asarray(Wc, np.float32)
    Cin = Wup.shape[2]
    Wout = Win * sx
    mats = {}
    for p in range(sy):
        deltas = {0}
        if p == 0:
            deltas.add(-1)
        if p == sy - 1:
            deltas.add(1)
        for d in sorted(deltas):
            M = np.zeros((Win * Cin, n_out_cols), np.float32)
            y = sy + p  # representative interior row
            i_t = y // sy + d
            nz = False
            for dy in (-1, 0, 1):
                yp = y + dy
                if yp // sy != i_t:
                    continue
                py = yp % sy
                for x in range(Wout):
                    for dx in (-1, 0, 1):
                        xp = x + dx
                        if xp < 0 or xp >= Wout:
                            continue
                        j, qx = divmod(xp, sx)
                        # conv_transpose (transpose_kernel=False) applies the
                        # spatially mirrored kernel per phase
                        CC = Wup[sy - 1 - py, sx - 1 - qx] @ Wc[dy + 1, dx + 1]
                        M[np.ix_(in_idx(j), out_idx(x))] += CC
                        nz = True
            if nz:
                mats[(p, d)] = M
    return mats


def build_host_matrices(W_lin, W_up1, W_c1, W_up2, W_c2, W_up3, W_c3):
    # L2 input = h natural ordering: feat = c*8 + j   (c<32, j<8)
    r2 = _fused_matrices(
        W_up1, W_c1, 5, 2, 8,
        in_idx=lambda j: np.arange(32) * 8 + j,
        out_idx=lambda x: x * 16 + np.arange(16),
        n_out_cols=256,
    )
    # L3 input ordering: feat = j*16 + c ; output feat = x*8 + o
    r3 = _fused_matrices(
        W_up2, W_c2, 5, 2, 16,
        in_idx=lambda j: j * 16 + np.arange(16),
        out_idx=lambda x: x * 8 + np.arange(8),
        n_out_cols=256,
    )
    # L4 input ordering: feat = j*8 + c ; output col = x*6 + o  (x-major:
    # this makes each 128-col group read only a 128-feat j-window)
    r4 = _fused_matrices(
        W_up3, W_c3, 2, 2, 32,
        in_idx=lambda j: j * 8 + np.arange(8),
        out_idx=lambda x: x * 6 + np.arange(6),
        n_out_cols=384,
    )
    return np.asarray(W_lin, np.float32), r2, r3, r4


def _contribs(p, i, n_in_rows, mats, sy):
    out = []
    for d in (-1, 0, 1):
        if (p, d) in mats and 0 <= i + d < n_in_rows:
            out.append((i + d, mats[(p, d)]))
    return out


def numpy_forward(latent, W_lin, b_lin, r2, r3, r4):
    """Pure-numpy forward through the fused matrices (golden check)."""
    def leaky(x):
        return np.where(x > 0, x, 0.01 * x)

    h = leaky(latent.astype(np.float32) @ W_lin + b_lin)  # [B, 256]
    rows = h[:, None, :]  # [B, 1, 256]
    for (mats, sy, n_in) in ((r2, 5, 1), (r3, 5, 5)):
        nrows = n_in * sy
        out = np.zeros((h.shape[0], nrows, 256), np.float32)
        for y in range(nrows):
            i, p = divmod(y, sy)
            for (src, M) in _contribs(p, i, n_in, mats, sy):
                out[:, y] += rows[:, src] @ M
        rows = leaky(out)
    out = np.zeros((h.shape[0], 50, 384), np.float32)
    for y in range(50):
        i, p = divmod(y, 2)
        for (src, M) in _contribs(p, i, 25, r4, 2):
            out[:, y] += rows[:, src] @ M
    out = np.maximum(out, 0.0)
    # cols are x-major (x*6+o): [B, 50, 64, 6] -> NCHW [B, 6, 50, 64]
    return out.reshape(-1, 50, 64, 6).transpose(0, 3, 1, 2)


# keys in fixed order for weight-tile indexing
R3_KEYS = [(0, -1), (0, 0), (1, 0), (2, 0), (3, 0), (4, 0), (4, 1)]
R4_KEYS = [(0, -1), (0, 0), (1, 0), (1, 1)]


def _key_contribs(p, i, n_in, keys):
    out = []
    for d in (-1, 0, 1):
        if (p, d) in keys and 0 <= i + d < n_in:
            out.append((i + d, keys.index((p, d))))
    return out


# ---------------------------------------------------------------- bass build
_CACHED = {}


def build_nc():
    nc = bacc.Bacc('TRN2', target_bir_lowering=False, debug=False,
                   num_devices=N_CORES)

    # w1 (cols 0:256) and latent (cols 256:256+BL) share one DMA; row 4 is
    # (b_lin | ones) so the bias rides the contraction for free.
    wlat = nc.declare_dram_parameter("wlat", [5, 256 + BL], F32R, isOutput=False)
    # w2: (y, kc, mc) 128x128 blocks of the 5 R2 row matrices
    w2 = nc.declare_dram_parameter("w2", [128, 5, 2, 2, 128], BF16, isOutput=False)
    # w3: (mat, kc, mc) 128x128 blocks of the 7 R3 matrices
    w3 = nc.declare_dram_parameter("w3", [128, 7, 2, 2, 128], BF16, isOutput=False)
    # w4: (mat, group) feat-window x 128-col blocks of the 4 R4 matrices
    w4 = nc.declare_dram_parameter("w4", [128, 4, 3, 128], BF16, isOutput=False)
    # out stored (b, y, x*6+o) in bf16; host casts + transposes to NCHW
    out = nc.declare_dram_parameter("out", [BL, 50, 384], BF16, isOutput=True)

    LR = mybir.ActivationFunctionType.Lrelu
    RELU = mybir.ActivationFunctionType.Relu

    with tile.TileContext(nc) as tc:
        with (
            tc.tile_pool(name="wpool", bufs=1) as wp,
            tc.tile_pool(name="acts", bufs=1) as ap,
            tc.tile_pool(name="ps", bufs=4, space=bass.MemorySpace.PSUM) as pp,
            tc.tile_pool(name="outp", bufs=10) as op,
        ):
            wlat_t = wp.tile([5, 256 + BL], F32R, tag="wlat")
            nc.sync.dma_start(out=wlat_t[:], in_=wlat[:])
            w2_t = wp.tile([128, 5, 2, 2, 128], BF16, tag="w2")
            nc.sync.dma_start(out=w2_t[:], in_=w2[:])
            w3_t = wp.tile([128, 7, 2, 2, 128], BF16, tag="w3")
            nc.sync.dma_start(out=w3_t[:], in_=w3[:])
            w4_t = wp.tile([128, 4, 3, 128], BF16, tag="w4")
            nc.sync.dma_start(out=w4_t[:], in_=w4[:])

            # PE p-state warmup: dependency-free matmuls (values are never
            # read) start the frequency ramp while the first weight DMA is
            # still in flight.
            warm = wp.tile([128, 128], BF16, tag="warm")
            nc.gpsimd.memset(warm[:], 0.0)
            wps = pp.tile([128, 2, BL], F32, tag="ps")
            for _ in range(18):
                nc.tensor.matmul(wps[:, 0, 0:128], warm[:], warm[:],
                                 start=True, stop=True)

            # ---- L1: h[256, B] = leaky(W_lin.T @ lat + b)
            x1 = ap.tile([128, 2, BL], BF16, tag="x1")
            ps = pp.tile([128, 2, BL], F32, tag="ps")
            for mc in range(2):
                nc.tensor.matmul(
                    ps[:, mc, :], wlat_t[:, bass.ts(mc, 128)], wlat_t[:, 256:256 + BL],
                    start=True, stop=True,
                )
            for mc in range(2):
                nc.scalar.activation(x1[:, mc, :], ps[:, mc, :], LR, alpha=0.01)

            # filler matmuls bridge the x1-activation / w2-DMA wait so the
            # PE busy stretch (and its frequency ramp) is never interrupted
            wps2 = pp.tile([128, 2, BL], F32, tag="ps")
            for _ in range(13):
                nc.tensor.matmul(wps2[:, 0, 0:128], warm[:], warm[:],
                                 start=True, stop=True)

            # ---- L2: 256 -> 1280 (5 rows x 256), input has 1 row
            x2 = ap.tile([128, 5, 2, BL], BF16, tag="x2")
            for y in range(5):
                ps = pp.tile([128, 2, BL], F32, tag="ps")
                for mc in range(2):
                    for kc in range(2):
                        nc.tensor.matmul(
                            ps[:, mc, :], w2_t[:, y, kc, mc, :], x1[:, kc, :],
                            start=(kc == 0), stop=(kc == 1),
                        )
                nc.scalar.activation(x2[:, y, :, :], ps[:, :, :], LR, alpha=0.01)

            # ---- L3: 1280 -> 6400 (25 rows x 256), 2-chunk contraction.
            # The pair-activation keeps ACT cheap; L4's middle feature
            # window (feats 80..176) is stitched by SBUF-to-SBUF DMA.
            x3 = ap.tile([128, 25, 2, BL], BF16, tag="x3")
            xm = ap.tile([96, 25, BL], BF16, tag="xm")  # feats 80..176
            for y in range(25):
                i, p = divmod(y, 5)
                cs = _key_contribs(p, i, 5, R3_KEYS)
                ps = pp.tile([128, 2, BL], F32, tag="ps")
                for mc in range(2):
                    n, tot = 0, len(cs) * 2
                    for (src, mi) in cs:
                        for kc in range(2):
                            nc.tensor.matmul(
                                ps[:, mc, :], w3_t[:, mi, kc, mc, :],
                                x2[:, src, kc, :],
                                start=(n == 0), stop=(n == tot - 1),
                            )
                            n += 1
                nc.scalar.activation(x3[:, y, :, :], ps[:, :, :], LR,
                                     alpha=0.01)
                if y % 5 == 4:
                    y0 = y - 4
                    nc.sync.dma_start(
                        out=xm[0:48, y0:y0 + 5, :],
                        in_=x3[80:128, y0:y0 + 5, 0, :])
                    nc.sync.dma_start(
                        out=xm[48:96, y0:y0 + 5, :],
                        in_=x3[0:48, y0:y0 + 5, 1, :])

            # ---- L4: 6400 -> 19200, batch-major psum pairs; relu split
            # ACT/DVE (GPSIMD cannot read PSUM); output flushed every other
            # pair, staggered by batch-block parity, so DMAs stay spread and
            # the kernel tail is a short transfer.
            relu_cycle = ("V", "A", "V")
            n_relu = 0
            for bb in range(4):
                flushes = {p for p in range(25) if p % 2 == bb % 2} | {24}
                ob, p0 = None, 0
                for pr in range(25):
                    if ob is None:
                        ob = op.tile([128, 4, 384], BF16, tag="ob",
                                     name=f"ob_{bb}_{pr}")
                        p0 = pr
                    ps = pp.tile([128, 2, 512], F32, tag="ps")
                    for yy in range(2):
                        y = 2 * pr + yy
                        i, p = divmod(y, 2)
                        cs = _key_contribs(p, i, 25, R4_KEYS)
                        for g in range(3):
                            n, tot = 0, len(cs)
                            for (src, mi) in cs:
                                if g == 0:
                                    lhs = x3[:, src, 0, bass.ts(bb, 128)]
                                    rhs = w4_t[:, mi, g, :]
                                elif g == 1:
                                    lhs = xm[:, src, bass.ts(bb, 128)]
                                    rhs = w4_t[0:96, mi, g, :]
                                else:
                                    lhs = x3[:, src, 1, bass.ts(bb, 128)]
                                    rhs = w4_t[:, mi, g, :]
                                nc.tensor.matmul(
                                    ps[:, yy, bass.ts(g, 128)], lhs, rhs,
                                    start=(n == 0), stop=(n == tot - 1),
                                )
                                n += 1
                    if bb == 3 and pr == 24:
                        eng = "A"  # fastest engine for the final relu
                    else:
                        eng = relu_cycle[n_relu % len(relu_cycle)]
                        n_relu += 1
                    r0 = 2 * (pr - p0)
                    dst = ob[:, r0:r0 + 2, :]
                    src_ap = ps[:, :, 0:384]
                    if eng == "A":
                        nc.scalar.activation(dst, src_ap, RELU)
                    else:
                        nc.vector.tensor_scalar_max(dst, src_ap, 0.0)
                    if pr in flushes:
                        nrow = 2 * (pr - p0) + 2
                        nc.sync.dma_start(
                            out=out[bass.ts(bb, 128), 2 * p0:2 * p0 + nrow, :],
                            in_=ob[:, 0:nrow, :],
                        )
                        ob = None
    nc.compile()
    return nc


# ---------------------------------------------------------------- entry
def kernel(**inputs):
    latent = np.asarray(inputs["latent_vector"], np.float32)
    W_lin, r2, r3, r4 = build_host_matrices(
        inputs["W_lin"], inputs["W_up1"], inputs["W_c1"],
        inputs["W_up2"], inputs["W_c2"], inputs["W_up3"], inputs["W_c3"],
    )
    b_lin = np.asarray(inputs["b_lin"], np.float32)

    if "nc" not in _CACHED:
        _CACHED["nc"] = build_nc()
    nc = _CACHED["nc"]

    # w2/w3: [K=128, n, kc, mc, 128] layouts
    def pack_blocks(mats_list):
        n = len(mats_list)
        t = np.zeros((128, n, 2, 2, 128), np.float32)
        for mi, M in enumerate(mats_list):
            for kc in range(2):
                for mc in range(2):
                    t[:, mi, kc, mc, :] = M[kc * 128:(kc + 1) * 128,
                                            mc * 128:(mc + 1) * 128]
        return np.ascontiguousarray(t.astype(NPBF))

    w2_host = pack_blocks([r2[(p, 0)] for p in range(5)])

    w3_host = pack_blocks([r3[k] for k in R3_KEYS])

    # w4: [128, 4, 3, 128]; group g takes rows L4_WIN[g], cols 128g..128(g+1)
    w4_host = np.zeros((128, 4, 3, 128), np.float32)
    for mi, k in enumerate(R4_KEYS):
        M = r4[k]
        for g, (r0, r1) in enumerate(L4_WIN):
            blk = M[r0:r1, g * 128:(g + 1) * 128]
            # sanity: all nonzeros of this col-group live inside the window
            rest = M[:, g * 128:(g + 1) * 128].copy()
            rest[r0:r1] = 0.0
            assert np.all(rest == 0.0), f"L4 window violated mat {k} group {g}"
            w4_host[:r1 - r0, mi, g, :] = blk
    w4_host = np.ascontiguousarray(w4_host.astype(NPBF))

    base = {"w2": w2_host, "w3": w3_host, "w4": w4_host}

    w1b = np.concatenate([W_lin, b_lin[None, :]], axis=0)  # [5, 256]
    in_maps = []
    for c in range(N_CORES):
        sh = latent[c * BL:(c + 1) * BL]
        lat1 = np.concatenate(
            [sh.T, np.ones((1, BL), np.float32)], axis=0)  # [5, BL]
        wlat = np.concatenate([w1b, lat1], axis=1)
        in_maps.append({**base,
                        "wlat": np.ascontiguousarray(wlat)})

    _CACHED["maps"] = in_maps
    res = run_bass_kernel_spmd(nc, in_maps, list(range(N_CORES)))
    outs = [
        np.asarray(r["out"], NPBF).astype(np.float32)
        .reshape(BL, 50, 64, 6).transpose(0, 3, 1, 2)
        for r in res.results
    ]
    return np.ascontiguousarray(np.concatenate(outs, axis=0))


if __name__ == "__main__":
    rng = np.random.default_rng(0)
    fake = {
        "latent_vector": rng.standard_normal((B, 4)).astype(np.float32),
        "W_lin": rng.standard_normal((4, 256)).astype(np.float32) * 0.5,
        "b_lin": np.zeros(256, np.float32),
        "W_up1": rng.standard_normal((5, 2, 32, 32)).astype(np.float32) * 0.1,
        "W_c1": rng.standard_normal((3, 3, 32, 16)).astype(np.float32) * 0.1,
        "W_up2": rng.standard_normal((5, 2, 16, 16)).astype(np.float32) * 0.1,
        "W_c2": rng.standard_normal((3, 3, 16, 8)).astype(np.float32) * 0.1,
        "W_up3": rng.standard_normal((2, 2, 8, 8)).astype(np.float32) * 0.1,
        "W_c3": rng.standard_normal((3, 3, 8, 6)).astype(np.float32) * 0.1,
    }
    o = kernel(**fake)
    print("kernel out", o.shape, o.dtype)
